# revision 1
# baseline (speedup 1.0000x reference)
"""Trainium2 Bass kernel v2 for autoregressive GRU sampling.

Design (vs v1: fp32 matmuls, 64-partition elementwise, serial engines):
  - Batch halves A/B packed vertically: state h as [128, 1024] per core
    (rows 0-63 = half A's hidden, rows 64-127 = half B), so every
    elementwise op uses all 128 partitions at half the free size.
  - Block-diagonal fp32r weights compute both halves' gates in one matmul
    (fp32r: ~tf32 precision, 1 cycle/row at N=512 vs fp32's 4).
  - Bit and bias contributions ride K=2 rank-1 matmuls that open each
    psum accumulation group (bits tile rows = bitA, bitB).
  - logit-space compare: host precomputes uhat = logit(u) - head_b; the
    bit is is_lt(uhat, head_w.h') straight out of the head matmul psum.
  - Elementwise chain split across engines: DVE (rg, npre, h'-add, cmp),
    GPSIMD (sub, z*d), ACT (sigmoid x2 with bias, tanh with bias).
  - Two 512-column chunks per site pipeline the engines across the
    sequential site recurrence.
"""

import numpy as np
from contextlib import ExitStack

HIDDEN = 64
N_SITES = 1024
BATCH = 16384
N_CORES = 8
B_LOCAL = BATCH // N_CORES   # 2048
HB = B_LOCAL // 2            # 1024 per packed half
CK = 512                     # matmul / elementwise chunk width
NCHUNK = HB // CK            # 2

_BUILD_CACHE = {}


def _build(n_sites: int, compile: bool = True):
    import concourse.bass as bass
    import concourse.bacc as bacc
    import concourse.tile as tile
    from concourse import mybir

    f32 = mybir.dt.float32
    f32r = mybir.dt.float32r
    bf16 = mybir.dt.bfloat16
    AF = mybir.ActivationFunctionType
    OP = mybir.AluOpType

    nc = bacc.Bacc()
    uhat_d = nc.dram_tensor("uhat", [n_sites, 4, HB], f32, kind="ExternalInput")
    wz_d = nc.dram_tensor("wz", [128, 128], f32r, kind="ExternalInput")
    wr_d = nc.dram_tensor("wr", [128, 128], f32r, kind="ExternalInput")
    wn_d = nc.dram_tensor("wn", [128, 128], f32r, kind="ExternalInput")
    wh_d = nc.dram_tensor("wh", [128, 4], f32r, kind="ExternalInput")
    wbz_d = nc.dram_tensor("wbz", [4, 128], bf16, kind="ExternalInput")
    wbr_d = nc.dram_tensor("wbr", [4, 128], bf16, kind="ExternalInput")
    wbn_d = nc.dram_tensor("wbn", [4, 128], bf16, kind="ExternalInput")
    bz_d = nc.dram_tensor("bz", [128, 1], f32, kind="ExternalInput")
    br_d = nc.dram_tensor("br", [128, 1], f32, kind="ExternalInput")
    bhn_d = nc.dram_tensor("bhn", [128, 1], f32, kind="ExternalInput")
    bin_d = nc.dram_tensor("bin", [128, 1], f32, kind="ExternalInput")
    bits_d = nc.dram_tensor("bits", [n_sites, 2, HB], bf16, kind="ExternalOutput")

    with ExitStack() as ctx:
        tc = ctx.enter_context(tile.TileContext(nc))
        const = ctx.enter_context(tc.tile_pool(name="const", bufs=1))
        work = ctx.enter_context(tc.tile_pool(name="work", bufs=3))
        upool = ctx.enter_context(tc.tile_pool(name="up", bufs=4))
        ps_z = ctx.enter_context(tc.tile_pool(name="psz", bufs=1, space="PSUM"))
        ps_r = ctx.enter_context(tc.tile_pool(name="psr", bufs=1, space="PSUM"))
        ps_n = ctx.enter_context(tc.tile_pool(name="psn", bufs=1, space="PSUM"))
        ps_ph = ctx.enter_context(tc.tile_pool(name="psp", bufs=1, space="PSUM"))

        # Weight/bias loads bounce through a DVE copy (keeps each consumer's
        # wait set small, same trick as v1).
        def load(name, shape, dt, dram):
            raw = const.tile(shape, dt, tag=f"raw_{name}", name=f"raw_{name}")
            dst = const.tile(shape, dt, tag=f"dst_{name}", name=f"dst_{name}")
            nc.sync.dma_start(raw[:], dram[:])
            nc.vector.tensor_copy(dst[:], raw[:])
            return dst

        wz = load("wz", [128, 128], f32r, wz_d)
        wr = load("wr", [128, 128], f32r, wr_d)
        wn = load("wn", [128, 128], f32r, wn_d)
        wh = load("wh", [128, 4], f32r, wh_d)
        wbz = load("wbz", [4, 128], bf16, wbz_d)
        wbr = load("wbr", [4, 128], bf16, wbr_d)
        wbn = load("wbn", [4, 128], bf16, wbn_d)
        bz = load("bz", [128, 1], f32, bz_d)
        br = load("br", [128, 1], f32, br_d)
        bhn = load("bhn", [128, 1], f32, bhn_d)
        bin_ = load("bin", [128, 1], f32, bin_d)

        # Ping-pong state tiles per chunk; bit tiles bf16 [4, CK] per chunk
        # (rows = bitA, bitA, bitB, bitB - duplicated for the hi/lo split
        # rank-1 weights).
        st = [[const.tile([128, CK], f32r, name=f"st{i}c{c}") for c in range(NCHUNK)]
              for i in range(2)]
        bt = [[const.tile([4, CK], bf16, name=f"bt{i}c{c}") for c in range(NCHUNK)]
              for i in range(2)]
        for row in st:
            for t_ in row:
                nc.vector.memset(t_[:].bitcast(f32), 0.0)
        for row in bt:
            for t_ in row:
                nc.vector.memset(t_[:], 0.0)

        for t in range(n_sites):
            for c in range(NCHUNK):
                cur, nxt = st[t % 2][c], st[(t + 1) % 2][c]
                bcur, bnxt = bt[t % 2][c], bt[(t + 1) % 2][c]
                cs = slice(c * CK, (c + 1) * CK)

                ust = upool.tile([4, CK], f32, tag=f"u{c}")
                nc.sync.dma_start(ust[:], uhat_d[t, :, cs])

                # Gate psums: bf16 hi/lo rank-1 bit terms open each group,
                # the block-diag f32r main accumulates on top.
                zp = ps_z.tile([128, CK], f32, tag=f"z{c}")
                nc.tensor.matmul(zp[:], wbz[:], bcur[:], start=True, stop=False,
                                 skip_group_check=True)
                nc.tensor.matmul(zp[:], wz[:], cur[:], start=False, stop=True,
                                 skip_group_check=True)
                rp = ps_r.tile([128, CK], f32, tag=f"r{c}")
                nc.tensor.matmul(rp[:], wbr[:], bcur[:], start=True, stop=False,
                                 skip_group_check=True)
                nc.tensor.matmul(rp[:], wr[:], cur[:], start=False, stop=True,
                                 skip_group_check=True)
                np_ = ps_n.tile([128, CK], f32, tag=f"n{c}")
                nc.tensor.matmul(np_[:], wn[:], cur[:], start=True, stop=False,
                                 skip_group_check=True)

                z_sb_t = work.tile([128, CK], f32, tag=f"zs{c}")
                nc.scalar.activation(z_sb_t[:], zp[:], AF.Sigmoid, bias=bz[:])
                r_sb_t = work.tile([128, CK], f32, tag=f"rs{c}")
                nc.scalar.activation(r_sb_t[:], rp[:], AF.Sigmoid, bias=br[:])
                z_sb = z_sb_t[:]
                r_sb = r_sb_t[:]

                # Off-critical-path pieces of the update h' = z'*nt + z*h
                # (z' = 1-z): both only need z and h, available early.
                zc = work.tile([128, CK], f32, tag=f"zc{c}")
                nc.gpsimd.tensor_scalar(zc[:], z_sb, -1.0, 1.0, OP.mult, OP.add)
                zh = work.tile([128, CK], f32, tag=f"zh{c}")
                nc.gpsimd.tensor_tensor(zh[:], z_sb, cur[:].bitcast(f32), OP.mult)

                # n-gate preact built up in the n psum itself: the stt writes
                # rg = (ghn + bhn)*r over ghn (has_written stays set from the
                # main matmul), then the gxn rank-1 ACCUMULATES bit*w_ihn on
                # top (start=False), so npre = rg + gxn lands in psum with no
                # extra DVE pass. tanh's bias port adds b_ihn.
                nc.vector.scalar_tensor_tensor(
                    np_[:], np_[:], bhn[:], r_sb, OP.add, OP.mult
                )
                nc.tensor.matmul(np_[:], wbn[:], bcur[:], start=False, stop=True,
                                 skip_group_check=True)
                nt = work.tile([128, CK], f32, tag=f"nt{c}")
                nc.scalar.activation(nt[:], np_[:], AF.Tanh, bias=bin_[:])

                # Critical tail: a = z'*nt ; h' = a + zh (both fast DVE ops).
                a_ = work.tile([128, CK], f32, tag=f"a{c}")
                nc.vector.tensor_tensor(a_[:], zc[:], nt[:], OP.mult)
                nc.vector.tensor_tensor(nxt[:], a_[:], zh[:], OP.add)

                # head: ph rows = (lA, lA, lB, lB); bit = (uhat < logit)
                php = ps_ph.tile([4, CK], f32, tag=f"ph{c}")
                nc.tensor.matmul(php[:], wh[:], nxt[:], start=True, stop=True)
                nc.vector.tensor_tensor(bnxt[0:4, :], ust[:], php[:], OP.is_lt)

                nc.sync.dma_start(bits_d[t, :, cs], bnxt[0:4:2, :])

    if compile:
        nc.compile()
    return nc


def _pack_inputs(u, w_ih, w_hh, b_ih, b_hh, head_w, head_b):
    H = HIDDEN
    w_ih = np.asarray(w_ih, np.float32)
    w_hh = np.asarray(w_hh, np.float32)
    b_ih = np.asarray(b_ih, np.float32)
    b_hh = np.asarray(b_hh, np.float32)
    head_w = np.asarray(head_w, np.float32)
    head_b = np.asarray(head_b, np.float32)

    # Gate order in reference: rows [0:H]=r, [H:2H]=z, [2H:3H]=n.
    Wr_, Wz_, Wn_ = w_hh[0:H], w_hh[H:2 * H], w_hh[2 * H:]
    wir, wiz, win = w_ih[0:H, 0], w_ih[H:2 * H, 0], w_ih[2 * H:, 0]

    def blk(W):  # block-diag [128,128], lhsT layout (K=h rows, M=gate cols)
        out = np.zeros((128, 128), np.float32)
        out[0:H, 0:H] = W.T
        out[H:128, H:128] = W.T
        return out

    wz = blk(Wz_)
    wr = blk(Wr_)
    wn = blk(Wn_)

    wh = np.zeros((128, 4), np.float32)
    wh[0:H, 0] = head_w[0]
    wh[0:H, 1] = head_w[0]
    wh[H:128, 2] = head_w[0]
    wh[H:128, 3] = head_w[0]

    import ml_dtypes

    def rank1(wvec):
        # [4,128] bf16, hi/lo split: rows (A_hi, A_lo, B_hi, B_lo) matched to
        # the bits tile rows (bitA, bitA, bitB, bitB).
        hi = wvec.astype(ml_dtypes.bfloat16).astype(np.float32)
        lo = (wvec - hi).astype(ml_dtypes.bfloat16).astype(np.float32)
        out = np.zeros((4, 128), np.float32)
        out[0, 0:H] = hi
        out[1, 0:H] = lo
        out[2, H:128] = hi
        out[3, H:128] = lo
        return out.astype(ml_dtypes.bfloat16)

    wbz = rank1(wiz)
    wbr = rank1(wir)
    wbn = rank1(win)

    def dup(v):  # [128,1] bias duplicated across halves
        return np.concatenate([v, v]).astype(np.float32)[:, None]

    bz = dup(b_ih[H:2 * H] + b_hh[H:2 * H])
    br = dup(b_ih[0:H] + b_hh[0:H])
    bhn = dup(b_hh[2 * H:])
    bin_ = dup(b_ih[2 * H:])

    # uhat = logit(u) - head_b, laid out [site, half-dup, col] per core
    # (rows uA, uA, uB, uB to match the 4-row ph/bits layout).
    u64 = np.asarray(u, np.float64)
    L = (np.log(u64) - np.log1p(-u64) - float(head_b[0])).astype(np.float32)  # [B,S]
    n_sites = u.shape[1]
    uhats = []
    for c in range(N_CORES):
        Lc = L[c * B_LOCAL:(c + 1) * B_LOCAL]          # [2048, S]
        Lr = Lc.reshape(2, HB, n_sites).transpose(2, 0, 1)  # [S, 2, HB]
        Ld = np.repeat(Lr, 2, axis=1)                  # [S, 4, HB] = A,A,B,B
        uhats.append(np.ascontiguousarray(Ld))
    return wz, wr, wn, wh, wbz, wbr, wbn, bz, br, bhn, bin_, uhats


def _in_maps(packed):
    wz, wr, wn, wh, wbz, wbr, wbn, bz, br, bhn, bin_, uhats = packed
    return [
        {
            "uhat": uhats[c], "wz": wz, "wr": wr, "wn": wn, "wh": wh,
            "wbz": wbz, "wbr": wbr, "wbn": wbn,
            "bz": bz, "br": br, "bhn": bhn, "bin": bin_,
        }
        for c in range(N_CORES)
    ]


def kernel(u, w_ih, w_hh, b_ih, b_hh, head_w, head_b):
    from concourse.bass_utils import run_bass_kernel_spmd

    u = np.asarray(u)
    n_sites = u.shape[1]
    if n_sites not in _BUILD_CACHE:
        _BUILD_CACHE[n_sites] = _build(n_sites)
    nc = _BUILD_CACHE[n_sites]

    packed = _pack_inputs(u, w_ih, w_hh, b_ih, b_hh, head_w, head_b)
    in_maps = _in_maps(packed)
    res = run_bass_kernel_spmd(nc, in_maps, list(range(N_CORES)))
    out = np.empty((BATCH, n_sites), np.int32)
    for c in range(N_CORES):
        bits = np.asarray(res.results[c]["bits"], np.float32)  # [S, 2, HB] {0,1}
        bc = bits.transpose(1, 2, 0).reshape(B_LOCAL, n_sites)
        out[c * B_LOCAL:(c + 1) * B_LOCAL] = (bc > 0.5).astype(np.int32)
    return out

